# revision 13
# baseline (speedup 1.0000x reference)
"""Trainium2 Bass kernel for EnhancedMetaWeightNetwork.

Full (unsharded) inputs in, full output out. Internally: 8 NeuronCores,
core c handles batch b = c // 2 and query-row half c % 2 (1024 rows).
Attention K/V are computed per-core for the full sequence of the core's
batch (duplicated across the 2 cores sharing a batch; no collectives).

Layout strategy on each core (fp32 storage, fp32r matmuls):
  - activations kept feature-major ("T" = [feature, token]) for the
    attention/projection chain, token-major for the meta-MLP layernorms.
  - scoresT [key, query] per head; softmax denominator via ones-matmul
    restream on the PE; reciprocal broadcast via rank-1 ones matmul.
  - importance lookup via indirect DMA gather from the vocab table.
  - K^T, Q^T, ctx^T spilled through DRAM scratch to fit SBUF.
"""

import numpy as np

H = 1024
NH = 8
HD = 128           # head dim
S = 2048           # keys / full sequence
SQ = 1024          # own query rows per core
MD = 256           # meta dim
MD2 = 128
VOCAB = 32000
MIN_W, MAX_W = 0.1, 5.0
LN_EPS = 1e-5
P = 128
INV_SQRT_HD = 1.0 / np.sqrt(np.float32(HD))

_CACHE = {}


def _build(stop=None):
    """stop in {None, "x", "qkv", "att", "o"}: truncate after that phase
    (debug bisection; a dummy zero output is written instead)."""
    import concourse.bass as bass
    import concourse.mybir as mybir
    import concourse.tile as tile
    from concourse import bacc
    from concourse.masks import make_identity

    f32 = mybir.dt.float32
    f32r = mybir.dt.float32r
    i32 = mybir.dt.int32
    OP = mybir.AluOpType
    ACT = mybir.ActivationFunctionType

    order = {"x": 0, "qkv": 1, "att": 2, "o": 3, "m1": 4, "m2": 5, None: 9}
    lvl = order[stop]

    nc = bacc.Bacc("TRN2", target_bir_lowering=False, debug=False,
                   enable_asserts=False, num_devices=8)

    # ---------------- DRAM parameters ----------------
    dp = nc.declare_dram_parameter
    hT = dp("hT", [H, S], f32, isOutput=False)            # hidden[b].T (own half first)
    pT = dp("pT", [H, S], f32, isOutput=False)            # pos_embed[0].T (own half first)
    wqT = dp("wqT", [H, H], f32r, isOutput=False)         # in_proj_w[0:H].T
    wkT = dp("wkT", [H, H], f32r, isOutput=False)
    wvT = dp("wvT", [H, H], f32r, isOutput=False)
    bq_c = dp("bq_c", [P, H // P], f32, isOutput=False)   # bias, partition-major
    bk_c = dp("bk_c", [P, H // P], f32, isOutput=False)
    bv_b = dp("bv_b", [P, H], f32, isOutput=False)        # bias bcast over partitions
    owT = dp("owT", [H, H], f32r, isOutput=False)         # out_w.T
    ob_c = dp("ob_c", [P, H // P], f32, isOutput=False)
    w1T = dp("w1T", [2 * H, MD], f32r, isOutput=False)
    b1_b = dp("b1_b", [P, MD], f32, isOutput=False)
    g1_b = dp("g1_b", [P, MD], f32, isOutput=False)
    be1_b = dp("be1_b", [P, MD], f32, isOutput=False)
    w2T = dp("w2T", [MD, MD2], f32r, isOutput=False)
    b2_b = dp("b2_b", [P, MD2], f32, isOutput=False)
    g2_b = dp("g2_b", [P, MD2], f32, isOutput=False)
    be2_b = dp("be2_b", [P, MD2], f32, isOutput=False)
    w3_b = dp("w3_b", [P, MD2], f32, isOutput=False)
    b3_c = dp("b3_c", [P, 1], f32, isOutput=False)
    kbias = dp("kbias", [P, S // P], f32, isOutput=False)  # attn mask bias per key
    maskf = dp("maskf", [P, SQ // P], f32, isOutput=False)
    tok = dp("tok", [SQ, 1], i32, isOutput=False)
    table = dp("table", [VOCAB, 1], f32, isOutput=False)
    out = dp("out", [SQ], f32, isOutput=True)

    NKT = S // P          # 16 key tiles
    NC8 = H // P          # 8 feature chunks
    NTT = SQ // P         # 8 own token tiles

    # DRAM scratch
    ktd = nc.dram_tensor("ktd", [H, S], f32r)      # K^T spill
    qtd = nc.dram_tensor("qtd", [H, SQ], f32r)     # Q^T spill
    ctd = nc.dram_tensor("ctd", [H, SQ], f32r)     # ctx^T spill

    with tile.TileContext(nc) as tc:
        with tc.tile_pool(name="const", bufs=1) as cst, \
             tc.tile_pool(name="xown", bufs=1) as xop:

            # ---------------- constants ----------------
            ones_f = cst.tile([P, P], f32, tag="ones_f")
            nc.any.memset(ones_f[:], 1.0)
            ones_r = cst.tile([P, P], f32r, tag="ones_r")
            nc.vector.tensor_copy(ones_r[:], ones_f[:])
            ident = cst.tile([P, P], f32, tag="ident")
            make_identity(nc, ident[:])
            eps_sb = cst.tile([P, 1], f32, tag="eps")
            nc.any.memset(eps_sb[:], LN_EPS)

            def cload(shape, tag, src):
                t = cst.tile(shape, f32, tag=tag)
                nc.sync.dma_start(t[:], src[:])
                return t

            kbias_sb = cload([P, NKT], "kbias", kbias)
            maskf_sb = cload([P, NTT], "maskf", maskf)
            b3_sb = cload([P, 1], "b3", b3_c)
            w3_sb = cload([P, MD2], "w3", w3_b)
            bq_sb = cload([P, NC8], "bq", bq_c)
            bk_sb = cload([P, NC8], "bk", bk_c)
            ob_sb = cload([P, NC8], "ob", ob_c)
            bv_sb = cload([P, H], "bv", bv_b)
            b1_sb = cload([P, MD], "b1", b1_b)
            g1_sb = cload([P, MD], "g1", g1_b)
            be1_sb = cload([P, MD], "be1", be1_b)
            b2_sb = cload([P, MD2], "b2", b2_b)
            g2_sb = cload([P, MD2], "g2", g2_b)
            be2_sb = cload([P, MD2], "be2", be2_b)

            if lvl < 5:
                dout = cst.tile([P, NTT], f32, tag="dout")
                nc.any.memset(dout[:], 0.0)
                nc.sync.dma_start(out[:].rearrange("(t p) -> p t", p=P), dout[:])

            x_own = xop.tile([P, NC8, SQ], f32r, tag="x_own")

            with tc.tile_pool(name="vpool", bufs=1) as vp:
                v_sb = vp.tile([P, NKT, H], f32r, tag="v")

                with tc.tile_pool(name="xoth", bufs=1) as xot, \
                     tc.tile_pool(name="ps_mm1", bufs=6, space="PSUM") as ps1:
                    x_oth = xot.tile([P, NC8, S - SQ], f32r, tag="x_oth")

                    # ---------- phase X ----------
                    with tc.tile_pool(name="tmpx", bufs=3) as tmp:
                        for c8 in range(NC8):
                            for half, xdst in ((0, x_own), (1, x_oth)):
                                ht = tmp.tile([P, SQ], f32, tag="ht")
                                pt = tmp.tile([P, SQ], f32, tag="pt")
                                cs = half * SQ
                                nc.sync.dma_start(ht[:], hT[c8 * P:(c8 + 1) * P, cs:cs + SQ])
                                nc.sync.dma_start(pt[:], pT[c8 * P:(c8 + 1) * P, cs:cs + SQ])
                                nc.vector.tensor_tensor(out=xdst[:, c8, :], in0=ht[:],
                                                        in1=pt[:], op=OP.add)

                    # ---------- phase Q ----------
                    with tc.tile_pool(name="wq", bufs=2) as wst, \
                         tc.tile_pool(name="stgq", bufs=4) as stg:
                        for dt in range(NC8 if lvl >= 1 else 0):
                            wq_sb = wst.tile([P, NC8, P], f32r, tag="wq")
                            nc.sync.dma_start(wq_sb[:], wqT[:, dt * P:(dt + 1) * P]
                                              .rearrange("(c p) n -> p c n", p=P))
                            psqs = [ps1.tile([P, 512], mybir.dt.float32, tag="mm512",
                                              name=f"psq{qb}") for qb in range(SQ // 512)]
                            for c8 in range(NC8):
                                for qb in range(SQ // 512):
                                    nc.tensor.matmul(psqs[qb][:],
                                                     lhsT=wq_sb[:, c8, :],
                                                     rhs=x_own[:, c8, qb * 512:(qb + 1) * 512],
                                                     start=(c8 == 0), stop=(c8 == NC8 - 1))
                            for qb in range(SQ // 512):
                                qstg = stg.tile([P, 512], f32r, tag="qstg")
                                nc.scalar.activation(qstg[:], psqs[qb][:], ACT.Identity,
                                                     bias=bq_sb[:, dt:dt + 1],
                                                     scale=INV_SQRT_HD)
                                nc.sync.dma_start(
                                    qtd[dt * P:(dt + 1) * P, qb * 512:(qb + 1) * 512],
                                    qstg[:])

                    # ---------- phase K ----------
                    with tc.tile_pool(name="wk", bufs=2) as wst, \
                         tc.tile_pool(name="stgk", bufs=4) as stg:
                        for dt in range(NC8 if lvl >= 1 else 0):
                            wk_sb = wst.tile([P, NC8, P], f32r, tag="wk")
                            nc.sync.dma_start(wk_sb[:], wkT[:, dt * P:(dt + 1) * P]
                                              .rearrange("(c p) n -> p c n", p=P))
                            psks = [ps1.tile([P, 512], mybir.dt.float32, tag="mm512",
                                              name=f"psk{sb}") for sb in range(S // 512)]
                            for c8 in range(NC8):
                                for sb in range(S // 512):
                                    if sb < SQ // 512:
                                        rhs = x_own[:, c8, sb * 512:(sb + 1) * 512]
                                    else:
                                        rhs = x_oth[:, c8, (sb - SQ // 512) * 512:
                                                    (sb - SQ // 512 + 1) * 512]
                                    nc.tensor.matmul(psks[sb][:], lhsT=wk_sb[:, c8, :],
                                                     rhs=rhs,
                                                     start=(c8 == 0), stop=(c8 == NC8 - 1))
                            for sb in range(S // 512):
                                kstg = stg.tile([P, 512], f32r, tag="kstg")
                                nc.scalar.activation(kstg[:], psks[sb][:], ACT.Identity,
                                                     bias=bk_sb[:, dt:dt + 1], scale=1.0)
                                nc.sync.dma_start(
                                    ktd[dt * P:(dt + 1) * P, sb * 512:(sb + 1) * 512],
                                    kstg[:])

                    # ---------- phase V ----------
                    with tc.tile_pool(name="wv", bufs=1) as wst:
                        if lvl >= 1:
                            wv_sb = wst.tile([P, NC8, H], f32r, tag="wv")
                            nc.sync.dma_start(wv_sb[:],
                                              wvT[:].rearrange("(c p) n -> p c n", p=P))
                        for tt in range(NKT if lvl >= 1 else 0):
                            psvs = [ps1.tile([P, 512], mybir.dt.float32, tag="mm512",
                                             name=f"psv{db}") for db in range(H // 512)]
                            for c8 in range(NC8):
                                if tt < NTT:
                                    lhsT = x_own[:, c8, tt * P:(tt + 1) * P]
                                else:
                                    lhsT = x_oth[:, c8, (tt - NTT) * P:(tt - NTT + 1) * P]
                                for db in range(H // 512):
                                    nc.tensor.matmul(psvs[db][:], lhsT=lhsT,
                                                     rhs=wv_sb[:, c8, db * 512:(db + 1) * 512],
                                                     start=(c8 == 0), stop=(c8 == NC8 - 1))
                            for db in range(H // 512):
                                nc.vector.tensor_tensor(
                                    out=v_sb[:, tt, db * 512:(db + 1) * 512],
                                    in0=psvs[db][:],
                                    in1=bv_sb[:, db * 512:(db + 1) * 512],
                                    op=OP.add)

                # ---------- attention (x_oth freed) ----------
                with tc.tile_pool(name="kqs", bufs=2) as kqs, \
                     tc.tile_pool(name="exps", bufs=3) as exps, \
                     tc.tile_pool(name="asml", bufs=2) as asml, \
                     tc.tile_pool(name="ps_sc", bufs=2, space="PSUM") as ps_sc, \
                     tc.tile_pool(name="ps_ctx", bufs=1, space="PSUM") as ps_ctx, \
                     tc.tile_pool(name="ps_dn", bufs=1, space="PSUM") as ps_dn:
                    for h in range(NH if lvl >= 2 else 0):
                        kt_h = kqs.tile([P, S], f32r, tag="kt_h")
                        nc.sync.dma_start(kt_h[:], ktd[h * P:(h + 1) * P, :])
                        qt_h = kqs.tile([P, SQ], f32r, tag="qt_h")
                        nc.sync.dma_start(qt_h[:], qtd[h * P:(h + 1) * P, :])

                        cps = ps_ctx.tile([P, SQ], mybir.dt.float32, tag="cps")
                        dn = ps_dn.tile([P, SQ], mybir.dt.float32, tag="dn")
                        for kt in range(NKT):
                            sc = ps_sc.tile([P, SQ], mybir.dt.float32, tag="sc")
                            for qb in range(SQ // 512):
                                nc.tensor.matmul(sc[:, qb * 512:(qb + 1) * 512],
                                                 lhsT=kt_h[:, kt * P:(kt + 1) * P],
                                                 rhs=qt_h[:, qb * 512:(qb + 1) * 512],
                                                 start=True, stop=True)
                            ex = exps.tile([P, SQ], f32r, tag="ex")
                            nc.scalar.activation(ex[:], sc[:], ACT.Exp,
                                                 bias=kbias_sb[:, kt:kt + 1], scale=1.0)
                            for qb in range(SQ // 512):
                                nc.tensor.matmul(cps[:, qb * 512:(qb + 1) * 512],
                                                 lhsT=v_sb[:, kt, h * P:(h + 1) * P],
                                                 rhs=ex[:, qb * 512:(qb + 1) * 512],
                                                 start=(kt == 0), stop=(kt == NKT - 1))
                                nc.tensor.matmul(dn[:, qb * 512:(qb + 1) * 512],
                                                 lhsT=ones_r[:],
                                                 rhs=ex[:, qb * 512:(qb + 1) * 512],
                                                 start=(kt == 0), stop=(kt == NKT - 1))
                        rcb = asml.tile([P, SQ], f32, tag="rcb")
                        with nc.allow_low_precision(reason="fp32 storage"):
                            nc.vector.reciprocal(rcb[:], dn[:])
                        cstg = asml.tile([P, SQ], f32r, tag="cstg")
                        nc.vector.tensor_tensor(out=cstg[:], in0=cps[:], in1=rcb[:],
                                                op=OP.mult)
                        nc.sync.dma_start(ctd[h * P:(h + 1) * P, :], cstg[:])

            # ---------- out-projection (v freed): attT feature-major ----------
            with tc.tile_pool(name="attT", bufs=1) as atp, \
                 tc.tile_pool(name="ps_mm2", bufs=6, space="PSUM") as ps2:
                attT = atp.tile([P, NC8, SQ], f32r, tag="attT")
                with tc.tile_pool(name="owp", bufs=1) as owp, \
                     tc.tile_pool(name="ctxq", bufs=2) as cxq:
                    if lvl >= 3:
                        ow_sb = owp.tile([P, NC8, H], f32r, tag="ow")
                        nc.sync.dma_start(ow_sb[:],
                                          owT[:].rearrange("(c p) n -> p c n", p=P))
                    if lvl >= 3:
                        ctx_qbs = []
                        for qb in range(SQ // 512):
                            ctx_qb = cxq.tile([P, NC8, 512], f32r, tag=f"ctx_qb{qb}")
                            nc.sync.dma_start(ctx_qb[:], ctd[:, qb * 512:(qb + 1) * 512]
                                              .rearrange("(c p) n -> p c n", p=P))
                            ctx_qbs.append(ctx_qb)
                        for dt in range(NC8):
                            psos = [ps2.tile([P, 512], mybir.dt.float32, tag="mm512",
                                             name=f"pso{qb}") for qb in range(SQ // 512)]
                            for c8 in range(NC8):
                                for qb in range(SQ // 512):
                                    nc.tensor.matmul(psos[qb][:],
                                                     lhsT=ow_sb[:, c8, dt * P:(dt + 1) * P],
                                                     rhs=ctx_qbs[qb][:, c8, :],
                                                     start=(c8 == 0), stop=(c8 == NC8 - 1))
                            for qb in range(SQ // 512):
                                nc.scalar.activation(attT[:, dt, qb * 512:(qb + 1) * 512],
                                                     psos[qb][:], ACT.Identity,
                                                     bias=ob_sb[:, dt:dt + 1], scale=1.0)

                # ---------- meta MLP ----------
                with tc.tile_pool(name="mw", bufs=1) as mw, \
                     tc.tile_pool(name="msml", bufs=3) as sml:
                    if lvl >= 4:
                        w1_sb = mw.tile([P, 2 * NC8, MD], f32r, tag="w1")
                        nc.sync.dma_start(w1_sb[:],
                                          w1T[:].rearrange("(c p) n -> p c n", p=P))
                        w2_sb = mw.tile([P, MD // P, MD2], f32r, tag="w2")
                        nc.sync.dma_start(w2_sb[:],
                                          w2T[:].rearrange("(c p) n -> p c n", p=P))
                        h1T = mw.tile([P, MD // P, SQ], f32r, tag="h1T")
                        res_sb = mw.tile([P, NTT], f32, tag="res")
                        imp_all = mw.tile([P, NTT], f32, tag="imp_all")
                        if lvl >= 9:
                            for tt in range(NTT):
                                it = sml.tile([P, 1], i32, tag="it")
                                nc.sync.dma_start(it[:], tok[tt * P:(tt + 1) * P, :])
                                nc.gpsimd.indirect_dma_start(
                                    out=imp_all[:, tt:tt + 1], out_offset=None,
                                    in_=table[:],
                                    in_offset=bass.IndirectOffsetOnAxis(ap=it[:, :1],
                                                                        axis=0))

                    def layer_norm_relu(hsrc_ps, bias_b, g_b, be_b, F, outname):
                        hb = sml.tile([P, F], f32, tag=outname + "_hb")
                        ssum = sml.tile([P, 1], f32, tag=outname + "_sum")
                        nc.vector.scalar_tensor_tensor(out=hb[:], in0=hsrc_ps[:],
                                                       scalar=1.0, in1=bias_b[:],
                                                       op0=OP.mult, op1=OP.add,
                                                       accum_out=ssum[:])
                        sq = sml.tile([P, F], f32, tag=outname + "_sq")
                        ssq = sml.tile([P, 1], f32, tag=outname + "_ssq")
                        nc.vector.scalar_tensor_tensor(out=sq[:], in0=hb[:], scalar=1.0,
                                                       in1=hb[:], op0=OP.mult,
                                                       op1=OP.mult, accum_out=ssq[:])
                        nm = sml.tile([P, 1], f32, tag=outname + "_nm")
                        nc.vector.tensor_scalar_mul(nm[:], ssum[:], -1.0 / F)
                        ex2 = sml.tile([P, 1], f32, tag=outname + "_ex2")
                        nc.vector.tensor_scalar_mul(ex2[:], ssq[:], 1.0 / F)
                        m2 = sml.tile([P, 1], f32, tag=outname + "_m2")
                        nc.vector.tensor_tensor(out=m2[:], in0=nm[:], in1=nm[:],
                                                op=OP.mult)
                        var = sml.tile([P, 1], f32, tag=outname + "_var")
                        nc.vector.tensor_tensor(out=var[:], in0=ex2[:], in1=m2[:],
                                                op=OP.subtract)
                        std = sml.tile([P, 1], f32, tag=outname + "_std")
                        nc.scalar.activation(std[:], var[:], ACT.Sqrt,
                                             bias=eps_sb[:, 0:1], scale=1.0)
                        rstd = sml.tile([P, 1], f32, tag=outname + "_rstd")
                        nc.vector.reciprocal(rstd[:], std[:])
                        t1 = sml.tile([P, F], f32, tag=outname + "_t1")
                        nc.vector.scalar_tensor_tensor(out=t1[:], in0=hb[:], scalar=nm[:],
                                                       op0=OP.add, in1=g_b[:], op1=OP.mult)
                        t2 = sml.tile([P, F], f32, tag=outname + "_t2")
                        nc.vector.scalar_tensor_tensor(out=t2[:], in0=t1[:],
                                                       scalar=rstd[:], op0=OP.mult,
                                                       in1=be_b[:], op1=OP.add)
                        hn = sml.tile([P, F], f32, tag=outname + "_hn")
                        nc.vector.tensor_scalar_max(hn[:], t2[:], 0.0)
                        return hn

                    for tt in range(NTT if lvl >= 4 else 0):
                        ph1_t = ps2.tile([P, 512], mybir.dt.float32, tag="mm512",
                                         name="ph1")
                        ph1 = ph1_t[:, :MD]
                        for c16 in range(2 * NC8):
                            if c16 < NC8:
                                lhsT = x_own[:, c16, tt * P:(tt + 1) * P]
                            else:
                                lhsT = attT[:, c16 - NC8, tt * P:(tt + 1) * P]
                            nc.tensor.matmul(ph1, lhsT=lhsT, rhs=w1_sb[:, c16, :],
                                             start=(c16 == 0), stop=(c16 == 2 * NC8 - 1))
                        h1n = layer_norm_relu(ph1, b1_sb, g1_sb, be1_sb, MD, "l1")
                        for ft in range(MD // P):
                            ptp_t = ps2.tile([P, 512], mybir.dt.float32, tag="mm512",
                                             name="ptp")
                            ptp = ptp_t[:, :P]
                            nc.tensor.transpose(ptp, h1n[:, ft * P:(ft + 1) * P], ident[:])
                            nc.vector.tensor_copy(h1T[:, ft, tt * P:(tt + 1) * P], ptp)

                        if lvl < 5:
                            continue
                        ph2_t = ps2.tile([P, 512], mybir.dt.float32, tag="mm512",
                                         name="ph2")
                        ph2 = ph2_t[:, :MD2]
                        for ft in range(MD // P):
                            nc.tensor.matmul(ph2, lhsT=h1T[:, ft, tt * P:(tt + 1) * P],
                                             rhs=w2_sb[:, ft, :],
                                             start=(ft == 0), stop=(ft == MD // P - 1))
                        h2n = layer_norm_relu(ph2, b2_sb, g2_sb, be2_sb, MD2, "l2")
                        dsc = sml.tile([P, MD2], f32, tag="dsc")
                        base0 = sml.tile([P, 1], f32, tag="base0")
                        nc.vector.scalar_tensor_tensor(out=dsc[:], in0=h2n[:],
                                                       scalar=1.0, in1=w3_sb[:],
                                                       op0=OP.mult, op1=OP.mult,
                                                       accum_out=base0[:])
                        base = sml.tile([P, 1], f32, tag="base")
                        nc.vector.tensor_tensor(out=base[:], in0=base0[:],
                                                in1=b3_sb[:, 0:1], op=OP.add)
                        imp1 = sml.tile([P, 1], f32, tag="imp1")
                        nc.vector.tensor_scalar_add(imp1[:], imp_all[:, tt:tt + 1], 1.0)
                        wgt = sml.tile([P, 1], f32, tag="wgt")
                        nc.vector.tensor_tensor(out=wgt[:], in0=base[:], in1=imp1[:],
                                                op=OP.mult)
                        wcl = sml.tile([P, 1], f32, tag="wcl")
                        nc.vector.tensor_scalar(wcl[:], wgt[:], MAX_W, MIN_W,
                                                op0=OP.min, op1=OP.max)
                        nc.vector.tensor_tensor(out=res_sb[:, tt:tt + 1], in0=wcl[:],
                                                in1=maskf_sb[:, tt:tt + 1], op=OP.mult)

                    if lvl >= 5:
                        nc.sync.dma_start(out[:].rearrange("(t p) -> p t", p=P),
                                          res_sb[:])

    nc.compile()
    return nc


def _get_program():
    import os
    stop = os.environ.get("KB_STOP") or None
    key = ("nc", stop)
    if key not in _CACHE:
        _CACHE[key] = _build(stop)
    return _CACHE[key]


def _prep_in_maps(inputs):
    hidden = np.ascontiguousarray(np.asarray(inputs["hidden_states"], dtype=np.float32))
    token_ids = np.asarray(inputs["token_ids"], dtype=np.int32)
    mask = np.asarray(inputs["attention_mask"]).astype(bool)
    pos = np.asarray(inputs["pos_embed"], dtype=np.float32)
    in_proj_w = np.asarray(inputs["in_proj_w"], dtype=np.float32)
    in_proj_b = np.asarray(inputs["in_proj_b"], dtype=np.float32)
    out_w = np.asarray(inputs["out_w"], dtype=np.float32)
    out_b = np.asarray(inputs["out_b"], dtype=np.float32)
    w1 = np.asarray(inputs["w1"], dtype=np.float32)
    b1 = np.asarray(inputs["b1"], dtype=np.float32)
    g1 = np.asarray(inputs["g1"], dtype=np.float32)
    beta1 = np.asarray(inputs["beta1"], dtype=np.float32)
    w2 = np.asarray(inputs["w2"], dtype=np.float32)
    b2 = np.asarray(inputs["b2"], dtype=np.float32)
    g2 = np.asarray(inputs["g2"], dtype=np.float32)
    beta2 = np.asarray(inputs["beta2"], dtype=np.float32)
    w3 = np.asarray(inputs["w3"], dtype=np.float32)
    b3 = np.asarray(inputs["b3"], dtype=np.float32)
    table = np.asarray(inputs["importance_table"], dtype=np.float32)

    B, S_, H_ = hidden.shape
    assert (B, S_, H_) == (4, S, H), (B, S_, H_)

    posT = np.ascontiguousarray(pos[0].T)                      # [H, S]
    wqT = np.ascontiguousarray(in_proj_w[0:H].T)               # [H, H]
    wkT = np.ascontiguousarray(in_proj_w[H:2 * H].T)
    wvT = np.ascontiguousarray(in_proj_w[2 * H:3 * H].T)
    bq = in_proj_b[0:H]
    bk = in_proj_b[H:2 * H]
    bv = in_proj_b[2 * H:3 * H]
    owT = np.ascontiguousarray(out_w.T)
    w1T = np.ascontiguousarray(w1.T)                           # [2H, MD]
    w2T = np.ascontiguousarray(w2.T)                           # [MD, MD2]

    def cmaj(v):   # [H] -> [128, H/128] partition-major (column dt holds v[dt*128+p])
        return np.ascontiguousarray(v.reshape(-1, P).T)

    def bcast(v):  # [F] -> [128, F]
        return np.ascontiguousarray(np.broadcast_to(v[None, :], (P, v.shape[0])))

    shared = {
        "wqT": wqT, "wkT": wkT, "wvT": wvT,
        "bq_c": cmaj(bq), "bk_c": cmaj(bk), "bv_b": bcast(bv),
        "owT": owT, "ob_c": cmaj(out_b),
        "w1T": w1T, "b1_b": bcast(b1), "g1_b": bcast(g1), "be1_b": bcast(beta1),
        "w2T": w2T, "b2_b": bcast(b2), "g2_b": bcast(g2), "be2_b": bcast(beta2),
        "w3_b": bcast(w3[0]), "b3_c": np.full((P, 1), b3[0], dtype=np.float32),
        "table": np.ascontiguousarray(table[:, None]),
    }

    in_maps = []
    for c in range(8):
        b = c // 2
        half = c % 2
        own = slice(half * SQ, (half + 1) * SQ)
        oth = slice((1 - half) * SQ, (2 - half) * SQ)
        hT_b = hidden[b].T                                     # [H, S] view
        # arrange so own half occupies columns [0, SQ)
        hT_arr = np.ascontiguousarray(
            np.concatenate([hT_b[:, own], hT_b[:, oth]], axis=1))
        pT_arr = np.ascontiguousarray(
            np.concatenate([posT[:, own], posT[:, oth]], axis=1))
        kb = np.where(mask[b], 0.0, -1e9).astype(np.float32)
        kb_arr = np.concatenate([kb[own], kb[oth]])            # match column remap
        m = {
            "hT": hT_arr, "pT": pT_arr,
            "kbias": np.ascontiguousarray(kb_arr.reshape(-1, P).T),
            "maskf": np.ascontiguousarray(
                mask[b, own].astype(np.float32).reshape(-1, P).T),
            "tok": np.ascontiguousarray(token_ids[b, own][:, None]),
        }
        m.update(shared)
        in_maps.append(m)
    return in_maps


def _assemble(res):
    full = np.zeros((4, S), dtype=np.float32)
    for c in range(8):
        b = c // 2
        half = c % 2
        full[b, half * SQ:(half + 1) * SQ] = res.results[c]["out"]
    return full


def kernel(**inputs) -> np.ndarray:
    from concourse.bass_utils import run_bass_kernel_spmd
    in_maps = _prep_in_maps(inputs)
    nc = _get_program()
    res = run_bass_kernel_spmd(nc, in_maps, list(range(8)))
    return _assemble(res)


def run_traced(inputs, **kwargs):
    from concourse.bass_utils import run_bass_kernel_spmd
    in_maps = _prep_in_maps(inputs)
    nc = _get_program()
    return run_bass_kernel_spmd(nc, in_maps, list(range(8)), trace=True, **kwargs)


# revision 14
# speedup vs baseline: 1.1064x; 1.1064x over previous
"""Trainium2 Bass kernel for EnhancedMetaWeightNetwork.

Full (unsharded) inputs in, full output out. Internally: 8 NeuronCores,
core c handles batch b = c // 2 and query-row half c % 2 (1024 rows).
Attention K/V are computed per-core for the full sequence of the core's
batch (duplicated across the 2 cores sharing a batch; no collectives).

Layout strategy on each core (fp32 storage, fp32r matmuls):
  - activations kept feature-major ("T" = [feature, token]) for the
    attention/projection chain, token-major for the meta-MLP layernorms.
  - scoresT [key, query] per head; softmax denominator via ones-matmul
    restream on the PE; reciprocal broadcast via rank-1 ones matmul.
  - importance lookup via indirect DMA gather from the vocab table.
  - K^T, Q^T, ctx^T spilled through DRAM scratch to fit SBUF.
"""

import numpy as np

H = 1024
NH = 8
HD = 128           # head dim
S = 2048           # keys / full sequence
SQ = 1024          # own query rows per core
MD = 256           # meta dim
MD2 = 128
VOCAB = 32000
MIN_W, MAX_W = 0.1, 5.0
LN_EPS = 1e-5
P = 128
INV_SQRT_HD = 1.0 / np.sqrt(np.float32(HD))

_CACHE = {}


def _build(stop=None):
    """stop in {None, "x", "qkv", "att", "o"}: truncate after that phase
    (debug bisection; a dummy zero output is written instead)."""
    import concourse.bass as bass
    import concourse.mybir as mybir
    import concourse.tile as tile
    from concourse import bacc
    from concourse.masks import make_identity

    f32 = mybir.dt.float32
    f32r = mybir.dt.float32r
    i32 = mybir.dt.int32
    OP = mybir.AluOpType
    ACT = mybir.ActivationFunctionType

    order = {"x": 0, "qkv": 1, "att": 2, "o": 3, "m1": 4, "m2": 5, None: 9}
    lvl = order[stop]

    nc = bacc.Bacc("TRN2", target_bir_lowering=False, debug=False,
                   enable_asserts=False, num_devices=8)

    # ---------------- DRAM parameters ----------------
    dp = nc.declare_dram_parameter
    hT = dp("hT", [H, S], f32, isOutput=False)            # hidden[b].T (own half first)
    pT = dp("pT", [H, S], f32, isOutput=False)            # pos_embed[0].T (own half first)
    wqT = dp("wqT", [H, H], f32r, isOutput=False)         # in_proj_w[0:H].T
    wkT = dp("wkT", [H, H], f32r, isOutput=False)
    wvT = dp("wvT", [H, H], f32r, isOutput=False)
    bq_c = dp("bq_c", [P, H // P], f32, isOutput=False)   # bias, partition-major
    bk_c = dp("bk_c", [P, H // P], f32, isOutput=False)
    bv_b = dp("bv_b", [P, H], f32, isOutput=False)        # bias bcast over partitions
    owT = dp("owT", [H, H], f32r, isOutput=False)         # out_w.T
    ob_c = dp("ob_c", [P, H // P], f32, isOutput=False)
    w1T = dp("w1T", [2 * H, MD], f32r, isOutput=False)
    b1_b = dp("b1_b", [P, MD], f32, isOutput=False)
    g1_b = dp("g1_b", [P, MD], f32, isOutput=False)
    be1_b = dp("be1_b", [P, MD], f32, isOutput=False)
    w2T = dp("w2T", [MD, MD2], f32r, isOutput=False)
    b2_b = dp("b2_b", [P, MD2], f32, isOutput=False)
    g2_b = dp("g2_b", [P, MD2], f32, isOutput=False)
    be2_b = dp("be2_b", [P, MD2], f32, isOutput=False)
    w3_b = dp("w3_b", [P, MD2], f32, isOutput=False)
    b3_c = dp("b3_c", [P, 1], f32, isOutput=False)
    kbias = dp("kbias", [P, S // P], f32, isOutput=False)  # attn mask bias per key
    maskf = dp("maskf", [P, SQ // P], f32, isOutput=False)
    tok = dp("tok", [SQ, 1], i32, isOutput=False)
    table = dp("table", [VOCAB, 1], f32, isOutput=False)
    out = dp("out", [SQ], f32, isOutput=True)

    NKT = S // P          # 16 key tiles
    NC8 = H // P          # 8 feature chunks
    NTT = SQ // P         # 8 own token tiles

    # DRAM scratch
    ktd = nc.dram_tensor("ktd", [H, S], f32r)      # K^T spill
    qtd = nc.dram_tensor("qtd", [H, SQ], f32r)     # Q^T spill
    ctd = nc.dram_tensor("ctd", [H, SQ], f32r)     # ctx^T spill

    with tile.TileContext(nc) as tc:
        with tc.tile_pool(name="const", bufs=1) as cst, \
             tc.tile_pool(name="xown", bufs=1) as xop:

            # ---------------- constants ----------------
            ones_f = cst.tile([P, P], f32, tag="ones_f")
            nc.any.memset(ones_f[:], 1.0)
            ones_r = cst.tile([P, P], f32r, tag="ones_r")
            nc.vector.tensor_copy(ones_r[:], ones_f[:])
            ident = cst.tile([P, P], f32, tag="ident")
            make_identity(nc, ident[:])
            eps_sb = cst.tile([P, 1], f32, tag="eps")
            nc.any.memset(eps_sb[:], LN_EPS)

            def cload(shape, tag, src):
                t = cst.tile(shape, f32, tag=tag)
                nc.sync.dma_start(t[:], src[:])
                return t

            kbias_sb = cload([P, NKT], "kbias", kbias)
            maskf_sb = cload([P, NTT], "maskf", maskf)
            b3_sb = cload([P, 1], "b3", b3_c)
            w3_sb = cload([P, MD2], "w3", w3_b)
            bq_sb = cload([P, NC8], "bq", bq_c)
            bk_sb = cload([P, NC8], "bk", bk_c)
            ob_sb = cload([P, NC8], "ob", ob_c)
            bv_sb = cload([P, H], "bv", bv_b)
            b1_sb = cload([P, MD], "b1", b1_b)
            g1_sb = cload([P, MD], "g1", g1_b)
            be1_sb = cload([P, MD], "be1", be1_b)
            b2_sb = cload([P, MD2], "b2", b2_b)
            g2_sb = cload([P, MD2], "g2", g2_b)
            be2_sb = cload([P, MD2], "be2", be2_b)

            if lvl < 5:
                dout = cst.tile([P, NTT], f32, tag="dout")
                nc.any.memset(dout[:], 0.0)
                nc.sync.dma_start(out[:].rearrange("(t p) -> p t", p=P), dout[:])

            x_own = xop.tile([P, NC8, SQ], f32r, tag="x_own")

            with tc.tile_pool(name="vpool", bufs=1) as vp:
                v_sb = vp.tile([P, NKT, H], f32r, tag="v")

                with tc.tile_pool(name="xoth", bufs=1) as xot, \
                     tc.tile_pool(name="ps_mm1", bufs=6, space="PSUM") as ps1:
                    x_oth = xot.tile([P, NC8, S - SQ], f32r, tag="x_oth")

                    # ---------- phase X ----------
                    with tc.tile_pool(name="tmpx", bufs=3) as tmp:
                        for c8 in range(NC8):
                            for half, xdst in ((0, x_own), (1, x_oth)):
                                ht = tmp.tile([P, SQ], f32, tag="ht")
                                pt = tmp.tile([P, SQ], f32, tag="pt")
                                cs = half * SQ
                                nc.sync.dma_start(ht[:], hT[c8 * P:(c8 + 1) * P, cs:cs + SQ])
                                nc.sync.dma_start(pt[:], pT[c8 * P:(c8 + 1) * P, cs:cs + SQ])
                                nc.vector.tensor_tensor(out=xdst[:, c8, :], in0=ht[:],
                                                        in1=pt[:], op=OP.add)

                    # ---------- phase Q ----------
                    with tc.tile_pool(name="wq", bufs=2) as wst, \
                         tc.tile_pool(name="stgq", bufs=4) as stg:
                        for dt in range(NC8 if lvl >= 1 else 0):
                            wq_sb = wst.tile([P, NC8, P], f32r, tag="wq")
                            nc.sync.dma_start(wq_sb[:], wqT[:, dt * P:(dt + 1) * P]
                                              .rearrange("(c p) n -> p c n", p=P))
                            psqs = [ps1.tile([P, 512], mybir.dt.float32, tag="mm512",
                                              name=f"psq{qb}") for qb in range(SQ // 512)]
                            for c8 in range(NC8):
                                for qb in range(SQ // 512):
                                    nc.tensor.matmul(psqs[qb][:],
                                                     lhsT=wq_sb[:, c8, :],
                                                     rhs=x_own[:, c8, qb * 512:(qb + 1) * 512],
                                                     start=(c8 == 0), stop=(c8 == NC8 - 1))
                            for qb in range(SQ // 512):
                                qstg = stg.tile([P, 512], f32r, tag="qstg")
                                nc.scalar.activation(qstg[:], psqs[qb][:], ACT.Identity,
                                                     bias=bq_sb[:, dt:dt + 1],
                                                     scale=INV_SQRT_HD)
                                nc.sync.dma_start(
                                    qtd[dt * P:(dt + 1) * P, qb * 512:(qb + 1) * 512],
                                    qstg[:])

                    # ---------- phase K ----------
                    with tc.tile_pool(name="wk", bufs=2) as wst, \
                         tc.tile_pool(name="stgk", bufs=4) as stg:
                        for dt in range(NC8 if lvl >= 1 else 0):
                            wk_sb = wst.tile([P, NC8, P], f32r, tag="wk")
                            nc.sync.dma_start(wk_sb[:], wkT[:, dt * P:(dt + 1) * P]
                                              .rearrange("(c p) n -> p c n", p=P))
                            psks = [ps1.tile([P, 512], mybir.dt.float32, tag="mm512",
                                              name=f"psk{sb}") for sb in range(S // 512)]
                            for c8 in range(NC8):
                                for sb in range(S // 512):
                                    if sb < SQ // 512:
                                        rhs = x_own[:, c8, sb * 512:(sb + 1) * 512]
                                    else:
                                        rhs = x_oth[:, c8, (sb - SQ // 512) * 512:
                                                    (sb - SQ // 512 + 1) * 512]
                                    nc.tensor.matmul(psks[sb][:], lhsT=wk_sb[:, c8, :],
                                                     rhs=rhs,
                                                     start=(c8 == 0), stop=(c8 == NC8 - 1))
                            for sb in range(S // 512):
                                kstg = stg.tile([P, 512], f32r, tag="kstg")
                                nc.scalar.activation(kstg[:], psks[sb][:], ACT.Identity,
                                                     bias=bk_sb[:, dt:dt + 1], scale=1.0)
                                nc.sync.dma_start(
                                    ktd[dt * P:(dt + 1) * P, sb * 512:(sb + 1) * 512],
                                    kstg[:])

                    # ---------- phase V ----------
                    with tc.tile_pool(name="wv", bufs=1) as wst:
                        if lvl >= 1:
                            wv_sb = wst.tile([P, NC8, H], f32r, tag="wv")
                            nc.sync.dma_start(wv_sb[:],
                                              wvT[:].rearrange("(c p) n -> p c n", p=P))
                        for tt in range(NKT if lvl >= 1 else 0):
                            psvs = [ps1.tile([P, 512], mybir.dt.float32, tag="mm512",
                                             name=f"psv{db}") for db in range(H // 512)]
                            for c8 in range(NC8):
                                if tt < NTT:
                                    lhsT = x_own[:, c8, tt * P:(tt + 1) * P]
                                else:
                                    lhsT = x_oth[:, c8, (tt - NTT) * P:(tt - NTT + 1) * P]
                                for db in range(H // 512):
                                    nc.tensor.matmul(psvs[db][:], lhsT=lhsT,
                                                     rhs=wv_sb[:, c8, db * 512:(db + 1) * 512],
                                                     start=(c8 == 0), stop=(c8 == NC8 - 1))
                            for db in range(H // 512):
                                nc.vector.tensor_tensor(
                                    out=v_sb[:, tt, db * 512:(db + 1) * 512],
                                    in0=psvs[db][:],
                                    in1=bv_sb[:, db * 512:(db + 1) * 512],
                                    op=OP.add)

                # ---------- attention (x_oth freed) ----------
                with tc.tile_pool(name="kqs", bufs=2) as kqs, \
                     tc.tile_pool(name="exps", bufs=3) as exps, \
                     tc.tile_pool(name="asml", bufs=2) as asml, \
                     tc.tile_pool(name="ps_sc", bufs=2, space="PSUM") as ps_sc, \
                     tc.tile_pool(name="ps_ctx", bufs=1, space="PSUM") as ps_ctx, \
                     tc.tile_pool(name="ps_dn", bufs=1, space="PSUM") as ps_dn:
                    for h in range(NH if lvl >= 2 else 0):
                        kt_h = kqs.tile([P, S], f32r, tag="kt_h")
                        nc.sync.dma_start(kt_h[:], ktd[h * P:(h + 1) * P, :])
                        qt_h = kqs.tile([P, SQ], f32r, tag="qt_h")
                        nc.sync.dma_start(qt_h[:], qtd[h * P:(h + 1) * P, :])

                        cps = ps_ctx.tile([P, SQ], mybir.dt.float32, tag="cps")
                        dn = ps_dn.tile([P, SQ], mybir.dt.float32, tag="dn")
                        for kt in range(NKT):
                            sc = ps_sc.tile([P, SQ], mybir.dt.float32, tag="sc")
                            for qb in range(SQ // 512):
                                nc.tensor.matmul(sc[:, qb * 512:(qb + 1) * 512],
                                                 lhsT=kt_h[:, kt * P:(kt + 1) * P],
                                                 rhs=qt_h[:, qb * 512:(qb + 1) * 512],
                                                 start=True, stop=True)
                            ex = exps.tile([P, SQ], f32r, tag="ex")
                            nc.scalar.activation(ex[:], sc[:], ACT.Exp,
                                                 bias=kbias_sb[:, kt:kt + 1], scale=1.0)
                            for qb in range(SQ // 512):
                                nc.tensor.matmul(cps[:, qb * 512:(qb + 1) * 512],
                                                 lhsT=v_sb[:, kt, h * P:(h + 1) * P],
                                                 rhs=ex[:, qb * 512:(qb + 1) * 512],
                                                 start=(kt == 0), stop=(kt == NKT - 1))
                                nc.tensor.matmul(dn[:, qb * 512:(qb + 1) * 512],
                                                 lhsT=ones_r[:],
                                                 rhs=ex[:, qb * 512:(qb + 1) * 512],
                                                 start=(kt == 0), stop=(kt == NKT - 1))
                        cpsc = asml.tile([P, SQ], f32, tag="cpsc")
                        nc.vector.tensor_copy(cpsc[:], cps[:])
                        dnc = asml.tile([P, SQ], f32, tag="dnc")
                        nc.vector.tensor_copy(dnc[:], dn[:])
                        rcb = asml.tile([P, SQ], f32, tag="rcb")
                        with nc.allow_low_precision(reason="fp32 storage"):
                            nc.vector.reciprocal(rcb[:], dnc[:])
                        cstg = asml.tile([P, SQ], f32r, tag="cstg")
                        nc.vector.tensor_tensor(out=cstg[:], in0=cpsc[:], in1=rcb[:],
                                                op=OP.mult)
                        nc.sync.dma_start(ctd[h * P:(h + 1) * P, :], cstg[:])

            # ---------- out-projection (v freed): attT feature-major ----------
            with tc.tile_pool(name="attT", bufs=1) as atp, \
                 tc.tile_pool(name="ps_mm2", bufs=6, space="PSUM") as ps2:
                attT = atp.tile([P, NC8, SQ], f32r, tag="attT")
                with tc.tile_pool(name="owp", bufs=1) as owp, \
                     tc.tile_pool(name="ctxq", bufs=2) as cxq:
                    if lvl >= 3:
                        ow_sb = owp.tile([P, NC8, H], f32r, tag="ow")
                        for dt in range(NC8):
                            nc.sync.dma_start(ow_sb[:, :, dt * P:(dt + 1) * P],
                                              owT[:, dt * P:(dt + 1) * P]
                                              .rearrange("(c p) n -> p c n", p=P))
                    if lvl >= 3:
                        ctx_qbs = []
                        for qb in range(SQ // 512):
                            ctx_qb = cxq.tile([P, NC8, 512], f32r, tag=f"ctx_qb{qb}")
                            for c8 in range(NC8):
                                nc.sync.dma_start(
                                    ctx_qb[:, c8, :],
                                    ctd[c8 * P:(c8 + 1) * P, qb * 512:(qb + 1) * 512])
                            ctx_qbs.append(ctx_qb)
                        for dt in range(NC8):
                            psos = [ps2.tile([P, 512], mybir.dt.float32, tag="mm512",
                                             name=f"pso{qb}") for qb in range(SQ // 512)]
                            for c8 in range(NC8):
                                for qb in range(SQ // 512):
                                    nc.tensor.matmul(psos[qb][:],
                                                     lhsT=ow_sb[:, c8, dt * P:(dt + 1) * P],
                                                     rhs=ctx_qbs[qb][:, c8, :],
                                                     start=(c8 == 0), stop=(c8 == NC8 - 1))
                            for qb in range(SQ // 512):
                                nc.scalar.activation(attT[:, dt, qb * 512:(qb + 1) * 512],
                                                     psos[qb][:], ACT.Identity,
                                                     bias=ob_sb[:, dt:dt + 1], scale=1.0)

                # ---------- meta MLP ----------
                with tc.tile_pool(name="mw", bufs=1) as mw, \
                     tc.tile_pool(name="msml", bufs=3) as sml:
                    if lvl >= 4:
                        w1_sb = mw.tile([P, 2 * NC8, MD], f32r, tag="w1")
                        nc.sync.dma_start(w1_sb[:],
                                          w1T[:].rearrange("(c p) n -> p c n", p=P))
                        w2_sb = mw.tile([P, MD // P, MD2], f32r, tag="w2")
                        nc.sync.dma_start(w2_sb[:],
                                          w2T[:].rearrange("(c p) n -> p c n", p=P))
                        h1T = mw.tile([P, MD // P, SQ], f32r, tag="h1T")
                        res_sb = mw.tile([P, NTT], f32, tag="res")
                        imp_all = mw.tile([P, NTT], f32, tag="imp_all")
                        if lvl >= 9:
                            for tt in range(NTT):
                                it = sml.tile([P, 1], i32, tag="it")
                                nc.sync.dma_start(it[:], tok[tt * P:(tt + 1) * P, :])
                                nc.gpsimd.indirect_dma_start(
                                    out=imp_all[:, tt:tt + 1], out_offset=None,
                                    in_=table[:],
                                    in_offset=bass.IndirectOffsetOnAxis(ap=it[:, :1],
                                                                        axis=0))

                    def layer_norm_relu(hsrc_ps, bias_b, g_b, be_b, F, outname):
                        hb = sml.tile([P, F], f32, tag=outname + "_hb")
                        ssum = sml.tile([P, 1], f32, tag=outname + "_sum")
                        nc.vector.scalar_tensor_tensor(out=hb[:], in0=hsrc_ps[:],
                                                       scalar=1.0, in1=bias_b[:],
                                                       op0=OP.mult, op1=OP.add,
                                                       accum_out=ssum[:])
                        sq = sml.tile([P, F], f32, tag=outname + "_sq")
                        ssq = sml.tile([P, 1], f32, tag=outname + "_ssq")
                        nc.vector.scalar_tensor_tensor(out=sq[:], in0=hb[:], scalar=1.0,
                                                       in1=hb[:], op0=OP.mult,
                                                       op1=OP.mult, accum_out=ssq[:])
                        nm = sml.tile([P, 1], f32, tag=outname + "_nm")
                        nc.vector.tensor_scalar_mul(nm[:], ssum[:], -1.0 / F)
                        ex2 = sml.tile([P, 1], f32, tag=outname + "_ex2")
                        nc.vector.tensor_scalar_mul(ex2[:], ssq[:], 1.0 / F)
                        m2 = sml.tile([P, 1], f32, tag=outname + "_m2")
                        nc.vector.tensor_tensor(out=m2[:], in0=nm[:], in1=nm[:],
                                                op=OP.mult)
                        var = sml.tile([P, 1], f32, tag=outname + "_var")
                        nc.vector.tensor_tensor(out=var[:], in0=ex2[:], in1=m2[:],
                                                op=OP.subtract)
                        std = sml.tile([P, 1], f32, tag=outname + "_std")
                        nc.scalar.activation(std[:], var[:], ACT.Sqrt,
                                             bias=eps_sb[:, 0:1], scale=1.0)
                        rstd = sml.tile([P, 1], f32, tag=outname + "_rstd")
                        nc.vector.reciprocal(rstd[:], std[:])
                        t1 = sml.tile([P, F], f32, tag=outname + "_t1")
                        nc.vector.scalar_tensor_tensor(out=t1[:], in0=hb[:], scalar=nm[:],
                                                       op0=OP.add, in1=g_b[:], op1=OP.mult)
                        t2 = sml.tile([P, F], f32, tag=outname + "_t2")
                        nc.vector.scalar_tensor_tensor(out=t2[:], in0=t1[:],
                                                       scalar=rstd[:], op0=OP.mult,
                                                       in1=be_b[:], op1=OP.add)
                        hn = sml.tile([P, F], f32, tag=outname + "_hn")
                        nc.vector.tensor_scalar_max(hn[:], t2[:], 0.0)
                        return hn

                    for tt in range(NTT if lvl >= 4 else 0):
                        ph1_t = ps2.tile([P, 512], mybir.dt.float32, tag="mm512",
                                         name="ph1")
                        ph1 = ph1_t[:, :MD]
                        for c16 in range(2 * NC8):
                            if c16 < NC8:
                                lhsT = x_own[:, c16, tt * P:(tt + 1) * P]
                            else:
                                lhsT = attT[:, c16 - NC8, tt * P:(tt + 1) * P]
                            nc.tensor.matmul(ph1, lhsT=lhsT, rhs=w1_sb[:, c16, :],
                                             start=(c16 == 0), stop=(c16 == 2 * NC8 - 1))
                        h1n = layer_norm_relu(ph1, b1_sb, g1_sb, be1_sb, MD, "l1")
                        for ft in range(MD // P):
                            ptp_t = ps2.tile([P, 512], mybir.dt.float32, tag="mm512",
                                             name="ptp")
                            ptp = ptp_t[:, :P]
                            nc.tensor.transpose(ptp, h1n[:, ft * P:(ft + 1) * P], ident[:])
                            nc.vector.tensor_copy(h1T[:, ft, tt * P:(tt + 1) * P], ptp)

                        if lvl < 5:
                            continue
                        ph2_t = ps2.tile([P, 512], mybir.dt.float32, tag="mm512",
                                         name="ph2")
                        ph2 = ph2_t[:, :MD2]
                        for ft in range(MD // P):
                            nc.tensor.matmul(ph2, lhsT=h1T[:, ft, tt * P:(tt + 1) * P],
                                             rhs=w2_sb[:, ft, :],
                                             start=(ft == 0), stop=(ft == MD // P - 1))
                        h2n = layer_norm_relu(ph2, b2_sb, g2_sb, be2_sb, MD2, "l2")
                        dsc = sml.tile([P, MD2], f32, tag="dsc")
                        base0 = sml.tile([P, 1], f32, tag="base0")
                        nc.vector.scalar_tensor_tensor(out=dsc[:], in0=h2n[:],
                                                       scalar=1.0, in1=w3_sb[:],
                                                       op0=OP.mult, op1=OP.mult,
                                                       accum_out=base0[:])
                        base = sml.tile([P, 1], f32, tag="base")
                        nc.vector.tensor_tensor(out=base[:], in0=base0[:],
                                                in1=b3_sb[:, 0:1], op=OP.add)
                        imp1 = sml.tile([P, 1], f32, tag="imp1")
                        nc.vector.tensor_scalar_add(imp1[:], imp_all[:, tt:tt + 1], 1.0)
                        wgt = sml.tile([P, 1], f32, tag="wgt")
                        nc.vector.tensor_tensor(out=wgt[:], in0=base[:], in1=imp1[:],
                                                op=OP.mult)
                        wcl = sml.tile([P, 1], f32, tag="wcl")
                        nc.vector.tensor_scalar(wcl[:], wgt[:], MAX_W, MIN_W,
                                                op0=OP.min, op1=OP.max)
                        nc.vector.tensor_tensor(out=res_sb[:, tt:tt + 1], in0=wcl[:],
                                                in1=maskf_sb[:, tt:tt + 1], op=OP.mult)

                    if lvl >= 5:
                        nc.sync.dma_start(out[:].rearrange("(t p) -> p t", p=P),
                                          res_sb[:])

    nc.compile()
    return nc


def _get_program():
    import os
    stop = os.environ.get("KB_STOP") or None
    key = ("nc", stop)
    if key not in _CACHE:
        _CACHE[key] = _build(stop)
    return _CACHE[key]


def _prep_in_maps(inputs):
    hidden = np.ascontiguousarray(np.asarray(inputs["hidden_states"], dtype=np.float32))
    token_ids = np.asarray(inputs["token_ids"], dtype=np.int32)
    mask = np.asarray(inputs["attention_mask"]).astype(bool)
    pos = np.asarray(inputs["pos_embed"], dtype=np.float32)
    in_proj_w = np.asarray(inputs["in_proj_w"], dtype=np.float32)
    in_proj_b = np.asarray(inputs["in_proj_b"], dtype=np.float32)
    out_w = np.asarray(inputs["out_w"], dtype=np.float32)
    out_b = np.asarray(inputs["out_b"], dtype=np.float32)
    w1 = np.asarray(inputs["w1"], dtype=np.float32)
    b1 = np.asarray(inputs["b1"], dtype=np.float32)
    g1 = np.asarray(inputs["g1"], dtype=np.float32)
    beta1 = np.asarray(inputs["beta1"], dtype=np.float32)
    w2 = np.asarray(inputs["w2"], dtype=np.float32)
    b2 = np.asarray(inputs["b2"], dtype=np.float32)
    g2 = np.asarray(inputs["g2"], dtype=np.float32)
    beta2 = np.asarray(inputs["beta2"], dtype=np.float32)
    w3 = np.asarray(inputs["w3"], dtype=np.float32)
    b3 = np.asarray(inputs["b3"], dtype=np.float32)
    table = np.asarray(inputs["importance_table"], dtype=np.float32)

    B, S_, H_ = hidden.shape
    assert (B, S_, H_) == (4, S, H), (B, S_, H_)

    posT = np.ascontiguousarray(pos[0].T)                      # [H, S]
    wqT = np.ascontiguousarray(in_proj_w[0:H].T)               # [H, H]
    wkT = np.ascontiguousarray(in_proj_w[H:2 * H].T)
    wvT = np.ascontiguousarray(in_proj_w[2 * H:3 * H].T)
    bq = in_proj_b[0:H]
    bk = in_proj_b[H:2 * H]
    bv = in_proj_b[2 * H:3 * H]
    owT = np.ascontiguousarray(out_w.T)
    w1T = np.ascontiguousarray(w1.T)                           # [2H, MD]
    w2T = np.ascontiguousarray(w2.T)                           # [MD, MD2]

    def cmaj(v):   # [H] -> [128, H/128] partition-major (column dt holds v[dt*128+p])
        return np.ascontiguousarray(v.reshape(-1, P).T)

    def bcast(v):  # [F] -> [128, F]
        return np.ascontiguousarray(np.broadcast_to(v[None, :], (P, v.shape[0])))

    shared = {
        "wqT": wqT, "wkT": wkT, "wvT": wvT,
        "bq_c": cmaj(bq), "bk_c": cmaj(bk), "bv_b": bcast(bv),
        "owT": owT, "ob_c": cmaj(out_b),
        "w1T": w1T, "b1_b": bcast(b1), "g1_b": bcast(g1), "be1_b": bcast(beta1),
        "w2T": w2T, "b2_b": bcast(b2), "g2_b": bcast(g2), "be2_b": bcast(beta2),
        "w3_b": bcast(w3[0]), "b3_c": np.full((P, 1), b3[0], dtype=np.float32),
        "table": np.ascontiguousarray(table[:, None]),
    }

    in_maps = []
    for c in range(8):
        b = c // 2
        half = c % 2
        own = slice(half * SQ, (half + 1) * SQ)
        oth = slice((1 - half) * SQ, (2 - half) * SQ)
        hT_b = hidden[b].T                                     # [H, S] view
        # arrange so own half occupies columns [0, SQ)
        hT_arr = np.ascontiguousarray(
            np.concatenate([hT_b[:, own], hT_b[:, oth]], axis=1))
        pT_arr = np.ascontiguousarray(
            np.concatenate([posT[:, own], posT[:, oth]], axis=1))
        kb = np.where(mask[b], 0.0, -1e9).astype(np.float32)
        kb_arr = np.concatenate([kb[own], kb[oth]])            # match column remap
        m = {
            "hT": hT_arr, "pT": pT_arr,
            "kbias": np.ascontiguousarray(kb_arr.reshape(-1, P).T),
            "maskf": np.ascontiguousarray(
                mask[b, own].astype(np.float32).reshape(-1, P).T),
            "tok": np.ascontiguousarray(token_ids[b, own][:, None]),
        }
        m.update(shared)
        in_maps.append(m)
    return in_maps


def _assemble(res):
    full = np.zeros((4, S), dtype=np.float32)
    for c in range(8):
        b = c // 2
        half = c % 2
        full[b, half * SQ:(half + 1) * SQ] = res.results[c]["out"]
    return full


def kernel(**inputs) -> np.ndarray:
    from concourse.bass_utils import run_bass_kernel_spmd
    in_maps = _prep_in_maps(inputs)
    nc = _get_program()
    res = run_bass_kernel_spmd(nc, in_maps, list(range(8)))
    return _assemble(res)


def run_traced(inputs, **kwargs):
    from concourse.bass_utils import run_bass_kernel_spmd
    in_maps = _prep_in_maps(inputs)
    nc = _get_program()
    return run_bass_kernel_spmd(nc, in_maps, list(range(8)), trace=True, **kwargs)


# revision 16
# speedup vs baseline: 1.2191x; 1.1019x over previous
"""Trainium2 Bass kernel for EnhancedMetaWeightNetwork.

Full (unsharded) inputs in, full output out. Internally: 8 NeuronCores,
core c handles batch b = c // 2 and query-row half c % 2 (1024 rows).
Attention K/V are computed per-core for the full sequence of the core's
batch (duplicated across the 2 cores sharing a batch; no collectives).

Layout strategy on each core (fp32 storage, fp32r matmuls):
  - activations kept feature-major ("T" = [feature, token]) for the
    attention/projection chain, token-major for the meta-MLP layernorms.
  - scoresT [key, query] per head; softmax denominator via ones-matmul
    restream on the PE; reciprocal broadcast via rank-1 ones matmul.
  - importance lookup via indirect DMA gather from the vocab table.
  - K^T, Q^T, ctx^T spilled through DRAM scratch to fit SBUF.
"""

import numpy as np

H = 1024
NH = 8
HD = 128           # head dim
S = 2048           # keys / full sequence
SQ = 1024          # own query rows per core
MD = 256           # meta dim
MD2 = 128
VOCAB = 32000
MIN_W, MAX_W = 0.1, 5.0
LN_EPS = 1e-5
P = 128
INV_SQRT_HD = 1.0 / np.sqrt(np.float32(HD))

_CACHE = {}


def _build(stop=None):
    """stop in {None, "x", "qkv", "att", "o"}: truncate after that phase
    (debug bisection; a dummy zero output is written instead)."""
    import concourse.bass as bass
    import concourse.mybir as mybir
    import concourse.tile as tile
    from concourse import bacc
    from concourse.masks import make_identity

    f32 = mybir.dt.float32
    f32r = mybir.dt.float32r
    i32 = mybir.dt.int32
    OP = mybir.AluOpType
    ACT = mybir.ActivationFunctionType

    order = {"x": 0, "qkv": 1, "att": 2, "o": 3, "m1": 4, "m2": 5, None: 9}
    lvl = order[stop]

    nc = bacc.Bacc("TRN2", target_bir_lowering=False, debug=False,
                   enable_asserts=False, num_devices=8)

    # ---------------- DRAM parameters ----------------
    dp = nc.declare_dram_parameter
    hT = dp("hT", [H, S], f32, isOutput=False)            # hidden[b].T (own half first)
    pT = dp("pT", [H, S], f32, isOutput=False)            # pos_embed[0].T (own half first)
    wqT = dp("wqT", [H, H], f32r, isOutput=False)         # in_proj_w[0:H].T
    wkT = dp("wkT", [H, H], f32r, isOutput=False)
    wvT = dp("wvT", [H, H], f32r, isOutput=False)
    bq_c = dp("bq_c", [P, H // P], f32, isOutput=False)   # bias, partition-major
    bk_c = dp("bk_c", [P, H // P], f32, isOutput=False)
    bv_b = dp("bv_b", [P, H], f32, isOutput=False)        # bias bcast over partitions
    owT = dp("owT", [H, H], f32r, isOutput=False)         # out_w.T
    ob_c = dp("ob_c", [P, H // P], f32, isOutput=False)
    w1T = dp("w1T", [2 * H, MD], f32r, isOutput=False)
    b1_b = dp("b1_b", [P, MD], f32, isOutput=False)
    g1_b = dp("g1_b", [P, MD], f32, isOutput=False)
    be1_b = dp("be1_b", [P, MD], f32, isOutput=False)
    w2T = dp("w2T", [MD, MD2], f32r, isOutput=False)
    b2_b = dp("b2_b", [P, MD2], f32, isOutput=False)
    g2_b = dp("g2_b", [P, MD2], f32, isOutput=False)
    be2_b = dp("be2_b", [P, MD2], f32, isOutput=False)
    w3_b = dp("w3_b", [P, MD2], f32, isOutput=False)
    b3_c = dp("b3_c", [P, 1], f32, isOutput=False)
    kbias = dp("kbias", [P, S // P], f32, isOutput=False)  # attn mask bias per key
    maskf = dp("maskf", [P, SQ // P], f32, isOutput=False)
    tok = dp("tok", [SQ, 1], i32, isOutput=False)
    table = dp("table", [VOCAB, 1], f32, isOutput=False)
    out = dp("out", [SQ], f32, isOutput=True)

    NKT = S // P          # 16 key tiles
    NC8 = H // P          # 8 feature chunks
    NTT = SQ // P         # 8 own token tiles

    # DRAM scratch
    ktd = nc.dram_tensor("ktd", [H, S], f32r)      # K^T spill
    qtd = nc.dram_tensor("qtd", [H, SQ], f32r)     # Q^T spill
    ctd = nc.dram_tensor("ctd", [H, SQ], f32r)     # ctx^T spill

    with tile.TileContext(nc) as tc:
        with tc.tile_pool(name="const", bufs=1) as cst, \
             tc.tile_pool(name="xown", bufs=1) as xop:

            # ---------------- constants ----------------
            ones_f = cst.tile([P, P], f32, tag="ones_f")
            nc.any.memset(ones_f[:], 1.0)
            ones_r = cst.tile([P, P], f32r, tag="ones_r")
            nc.vector.tensor_copy(ones_r[:], ones_f[:])
            ident = cst.tile([P, P], f32, tag="ident")
            make_identity(nc, ident[:])
            eps_sb = cst.tile([P, 1], f32, tag="eps")
            nc.any.memset(eps_sb[:], LN_EPS)

            def cload(shape, tag, src):
                t = cst.tile(shape, f32, tag=tag)
                nc.sync.dma_start(t[:], src[:])
                return t

            kbias_sb = cload([P, NKT], "kbias", kbias)
            maskf_sb = cload([P, NTT], "maskf", maskf)
            b3_sb = cload([P, 1], "b3", b3_c)
            w3_sb = cload([P, MD2], "w3", w3_b)
            bq_sb = cload([P, NC8], "bq", bq_c)
            bk_sb = cload([P, NC8], "bk", bk_c)
            ob_sb = cload([P, NC8], "ob", ob_c)
            bv_sb = cload([P, H], "bv", bv_b)
            b1_sb = cload([P, MD], "b1", b1_b)
            g1_sb = cload([P, MD], "g1", g1_b)
            be1_sb = cload([P, MD], "be1", be1_b)
            b2_sb = cload([P, MD2], "b2", b2_b)
            g2_sb = cload([P, MD2], "g2", g2_b)
            be2_sb = cload([P, MD2], "be2", be2_b)

            if lvl < 5:
                dout = cst.tile([P, NTT], f32, tag="dout")
                nc.any.memset(dout[:], 0.0)
                nc.sync.dma_start(out[:].rearrange("(t p) -> p t", p=P), dout[:])

            x_own = xop.tile([P, NC8, SQ], f32r, tag="x_own")

            with tc.tile_pool(name="vpool", bufs=1) as vp:
                v_sb = vp.tile([P, NKT, H], f32r, tag="v")

                with tc.tile_pool(name="xoth", bufs=1) as xot, \
                     tc.tile_pool(name="ps_mm1", bufs=6, space="PSUM") as ps1:
                    x_oth = xot.tile([P, NC8, S - SQ], f32r, tag="x_oth")

                    # ---------- phase X ----------
                    with tc.tile_pool(name="tmpx", bufs=3) as tmp:
                        for c8 in range(NC8):
                            for half, xdst in ((0, x_own), (1, x_oth)):
                                ht = tmp.tile([P, SQ], f32, tag="ht")
                                pt = tmp.tile([P, SQ], f32, tag="pt")
                                cs = half * SQ
                                nc.sync.dma_start(ht[:], hT[c8 * P:(c8 + 1) * P, cs:cs + SQ])
                                nc.sync.dma_start(pt[:], pT[c8 * P:(c8 + 1) * P, cs:cs + SQ])
                                nc.vector.tensor_tensor(out=xdst[:, c8, :], in0=ht[:],
                                                        in1=pt[:], op=OP.add)

                    # ---------- phases Q/K/V ----------
                    with tc.tile_pool(name="wqkv", bufs=2) as wst, \
                         tc.tile_pool(name="wvp", bufs=1) as wvp, \
                         tc.tile_pool(name="stgqk", bufs=4) as stg:
                        if lvl >= 1:
                            wv_sb = wvp.tile([P, NC8, H], f32r, tag="wv")
                            for db in range(H // 512):
                                nc.sync.dma_start(
                                    wv_sb[:, :, db * 512:(db + 1) * 512],
                                    wvT[:, db * 512:(db + 1) * 512]
                                    .rearrange("(c p) n -> p c n", p=P))
                        for dt in range(NC8 if lvl >= 1 else 0):
                            wq_sb = wst.tile([P, NC8, P], f32r, tag="wq")
                            nc.sync.dma_start(wq_sb[:], wqT[:, dt * P:(dt + 1) * P]
                                              .rearrange("(c p) n -> p c n", p=P))
                            psqs = [ps1.tile([P, 512], mybir.dt.float32, tag="mm512",
                                              name=f"psq{qb}") for qb in range(SQ // 512)]
                            for c8 in range(NC8):
                                for qb in range(SQ // 512):
                                    nc.tensor.matmul(psqs[qb][:],
                                                     lhsT=wq_sb[:, c8, :],
                                                     rhs=x_own[:, c8, qb * 512:(qb + 1) * 512],
                                                     start=(c8 == 0), stop=(c8 == NC8 - 1))
                            for qb in range(SQ // 512):
                                qstg = stg.tile([P, 512], f32r, tag="qstg")
                                nc.scalar.activation(qstg[:], psqs[qb][:], ACT.Identity,
                                                     bias=bq_sb[:, dt:dt + 1],
                                                     scale=INV_SQRT_HD)
                                nc.sync.dma_start(
                                    qtd[dt * P:(dt + 1) * P, qb * 512:(qb + 1) * 512],
                                    qstg[:])

                    # ---------- phase K ----------
                        for dt in range(NC8 if lvl >= 1 else 0):
                            wk_sb = wst.tile([P, NC8, P], f32r, tag="wk")
                            nc.sync.dma_start(wk_sb[:], wkT[:, dt * P:(dt + 1) * P]
                                              .rearrange("(c p) n -> p c n", p=P))
                            psks = [ps1.tile([P, 512], mybir.dt.float32, tag="mm512",
                                              name=f"psk{sb}") for sb in range(S // 512)]
                            for c8 in range(NC8):
                                for sb in range(S // 512):
                                    if sb < SQ // 512:
                                        rhs = x_own[:, c8, sb * 512:(sb + 1) * 512]
                                    else:
                                        rhs = x_oth[:, c8, (sb - SQ // 512) * 512:
                                                    (sb - SQ // 512 + 1) * 512]
                                    nc.tensor.matmul(psks[sb][:], lhsT=wk_sb[:, c8, :],
                                                     rhs=rhs,
                                                     start=(c8 == 0), stop=(c8 == NC8 - 1))
                            for sb in range(S // 512):
                                kstg = stg.tile([P, 512], f32r, tag="kstg")
                                nc.scalar.activation(kstg[:], psks[sb][:], ACT.Identity,
                                                     bias=bk_sb[:, dt:dt + 1], scale=1.0)
                                nc.sync.dma_start(
                                    ktd[dt * P:(dt + 1) * P, sb * 512:(sb + 1) * 512],
                                    kstg[:])

                    # ---------- phase V ----------
                        for tt in range(NKT if lvl >= 1 else 0):
                            psvs = [ps1.tile([P, 512], mybir.dt.float32, tag="mm512",
                                             name=f"psv{db}") for db in range(H // 512)]
                            for c8 in range(NC8):
                                if tt < NTT:
                                    lhsT = x_own[:, c8, tt * P:(tt + 1) * P]
                                else:
                                    lhsT = x_oth[:, c8, (tt - NTT) * P:(tt - NTT + 1) * P]
                                for db in range(H // 512):
                                    nc.tensor.matmul(psvs[db][:], lhsT=lhsT,
                                                     rhs=wv_sb[:, c8, db * 512:(db + 1) * 512],
                                                     start=(c8 == 0), stop=(c8 == NC8 - 1))
                            for db in range(H // 512):
                                nc.vector.tensor_tensor(
                                    out=v_sb[:, tt, db * 512:(db + 1) * 512],
                                    in0=psvs[db][:],
                                    in1=bv_sb[:, db * 512:(db + 1) * 512],
                                    op=OP.add)

                # ---------- attention (x_oth freed) ----------
                with tc.tile_pool(name="kqs", bufs=2) as kqs, \
                     tc.tile_pool(name="exps", bufs=4) as exps, \
                     tc.tile_pool(name="asml", bufs=3) as asml, \
                     tc.tile_pool(name="ps_sc", bufs=4, space="PSUM") as ps_sc, \
                     tc.tile_pool(name="ps_ctx", bufs=2, space="PSUM") as ps_ctx, \
                     tc.tile_pool(name="ps_dn", bufs=2, space="PSUM") as ps_dn:
                    for h in range(NH if lvl >= 2 else 0):
                        kt_h = kqs.tile([P, S], f32r, tag="kt_h")
                        nc.sync.dma_start(kt_h[:], ktd[h * P:(h + 1) * P, :])
                        qt_h = kqs.tile([P, SQ], f32r, tag="qt_h")
                        nc.sync.dma_start(qt_h[:], qtd[h * P:(h + 1) * P, :])

                        for qb in range(SQ // 512):
                            qsl = slice(qb * 512, (qb + 1) * 512)
                            cps = ps_ctx.tile([P, 512], mybir.dt.float32, tag="cps")
                            dn = ps_dn.tile([P, 512], mybir.dt.float32, tag="dn")
                            for kt in range(NKT):
                                sc = ps_sc.tile([P, 512], mybir.dt.float32, tag="sc")
                                nc.tensor.matmul(sc[:],
                                                 lhsT=kt_h[:, kt * P:(kt + 1) * P],
                                                 rhs=qt_h[:, qsl],
                                                 start=True, stop=True)
                                ex = exps.tile([P, 512], f32r, tag="ex")
                                nc.scalar.activation(ex[:], sc[:], ACT.Exp,
                                                     bias=kbias_sb[:, kt:kt + 1],
                                                     scale=1.0)
                                nc.tensor.matmul(cps[:],
                                                 lhsT=v_sb[:, kt, h * P:(h + 1) * P],
                                                 rhs=ex[:],
                                                 start=(kt == 0), stop=(kt == NKT - 1))
                                nc.tensor.matmul(dn[:],
                                                 lhsT=ones_r[:],
                                                 rhs=ex[:],
                                                 start=(kt == 0), stop=(kt == NKT - 1))
                            cpsc = asml.tile([P, 512], f32, tag="cpsc")
                            nc.vector.tensor_copy(cpsc[:], cps[:])
                            dnc = asml.tile([P, 512], f32, tag="dnc")
                            nc.vector.tensor_copy(dnc[:], dn[:])
                            rcb = asml.tile([P, 512], f32, tag="rcb")
                            with nc.allow_low_precision(reason="fp32 storage"):
                                nc.vector.reciprocal(rcb[:], dnc[:])
                            cstg = asml.tile([P, 512], f32r, tag="cstg")
                            nc.vector.tensor_tensor(out=cstg[:], in0=cpsc[:],
                                                    in1=rcb[:], op=OP.mult)
                            nc.sync.dma_start(ctd[h * P:(h + 1) * P, qsl], cstg[:])

            # ---------- out-projection (v freed): attT feature-major ----------
            with tc.tile_pool(name="attT", bufs=1) as atp, \
                 tc.tile_pool(name="ps_mm2", bufs=6, space="PSUM") as ps2:
                attT = atp.tile([P, NC8, SQ], f32r, tag="attT")
                with tc.tile_pool(name="owp", bufs=1) as owp, \
                     tc.tile_pool(name="ctxq", bufs=2) as cxq:
                    if lvl >= 3:
                        ow_sb = owp.tile([P, NC8, H], f32r, tag="ow")
                        for dt in range(NC8):
                            nc.sync.dma_start(ow_sb[:, :, dt * P:(dt + 1) * P],
                                              owT[:, dt * P:(dt + 1) * P]
                                              .rearrange("(c p) n -> p c n", p=P))
                    if lvl >= 3:
                        ctx_qbs = []
                        for qb in range(SQ // 512):
                            ctx_qb = cxq.tile([P, NC8, 512], f32r, tag=f"ctx_qb{qb}")
                            for c8 in range(NC8):
                                nc.sync.dma_start(
                                    ctx_qb[:, c8, :],
                                    ctd[c8 * P:(c8 + 1) * P, qb * 512:(qb + 1) * 512])
                            ctx_qbs.append(ctx_qb)
                        for dt in range(NC8):
                            psos = [ps2.tile([P, 512], mybir.dt.float32, tag="mm512",
                                             name=f"pso{qb}") for qb in range(SQ // 512)]
                            for c8 in range(NC8):
                                for qb in range(SQ // 512):
                                    nc.tensor.matmul(psos[qb][:],
                                                     lhsT=ow_sb[:, c8, dt * P:(dt + 1) * P],
                                                     rhs=ctx_qbs[qb][:, c8, :],
                                                     start=(c8 == 0), stop=(c8 == NC8 - 1))
                            for qb in range(SQ // 512):
                                nc.scalar.activation(attT[:, dt, qb * 512:(qb + 1) * 512],
                                                     psos[qb][:], ACT.Identity,
                                                     bias=ob_sb[:, dt:dt + 1], scale=1.0)

                # ---------- meta MLP ----------
                with tc.tile_pool(name="mw", bufs=1) as mw, \
                     tc.tile_pool(name="msml", bufs=3) as sml:
                    if lvl >= 4:
                        w1_sb = mw.tile([P, 2 * NC8, MD], f32r, tag="w1")
                        nc.sync.dma_start(w1_sb[:],
                                          w1T[:].rearrange("(c p) n -> p c n", p=P))
                        w2_sb = mw.tile([P, MD // P, MD2], f32r, tag="w2")
                        nc.sync.dma_start(w2_sb[:],
                                          w2T[:].rearrange("(c p) n -> p c n", p=P))
                        h1T = mw.tile([P, MD // P, SQ], f32r, tag="h1T")
                        res_sb = mw.tile([P, NTT], f32, tag="res")
                        imp_all = mw.tile([P, NTT], f32, tag="imp_all")
                        if lvl >= 9:
                            for tt in range(NTT):
                                it = sml.tile([P, 1], i32, tag="it")
                                nc.sync.dma_start(it[:], tok[tt * P:(tt + 1) * P, :])
                                nc.gpsimd.indirect_dma_start(
                                    out=imp_all[:, tt:tt + 1], out_offset=None,
                                    in_=table[:],
                                    in_offset=bass.IndirectOffsetOnAxis(ap=it[:, :1],
                                                                        axis=0))

                    def layer_norm_relu(hsrc_ps, bias_b, g_b, be_b, F, outname):
                        hb = sml.tile([P, F], f32, tag=outname + "_hb")
                        ssum = sml.tile([P, 1], f32, tag=outname + "_sum")
                        nc.vector.scalar_tensor_tensor(out=hb[:], in0=hsrc_ps[:],
                                                       scalar=1.0, in1=bias_b[:],
                                                       op0=OP.mult, op1=OP.add,
                                                       accum_out=ssum[:])
                        sq = sml.tile([P, F], f32, tag=outname + "_sq")
                        ssq = sml.tile([P, 1], f32, tag=outname + "_ssq")
                        nc.vector.scalar_tensor_tensor(out=sq[:], in0=hb[:], scalar=1.0,
                                                       in1=hb[:], op0=OP.mult,
                                                       op1=OP.mult, accum_out=ssq[:])
                        nm = sml.tile([P, 1], f32, tag=outname + "_nm")
                        nc.vector.tensor_scalar_mul(nm[:], ssum[:], -1.0 / F)
                        ex2 = sml.tile([P, 1], f32, tag=outname + "_ex2")
                        nc.vector.tensor_scalar_mul(ex2[:], ssq[:], 1.0 / F)
                        m2 = sml.tile([P, 1], f32, tag=outname + "_m2")
                        nc.vector.tensor_tensor(out=m2[:], in0=nm[:], in1=nm[:],
                                                op=OP.mult)
                        var = sml.tile([P, 1], f32, tag=outname + "_var")
                        nc.vector.tensor_tensor(out=var[:], in0=ex2[:], in1=m2[:],
                                                op=OP.subtract)
                        std = sml.tile([P, 1], f32, tag=outname + "_std")
                        nc.scalar.activation(std[:], var[:], ACT.Sqrt,
                                             bias=eps_sb[:, 0:1], scale=1.0)
                        rstd = sml.tile([P, 1], f32, tag=outname + "_rstd")
                        nc.vector.reciprocal(rstd[:], std[:])
                        t1 = sml.tile([P, F], f32, tag=outname + "_t1")
                        nc.vector.scalar_tensor_tensor(out=t1[:], in0=hb[:], scalar=nm[:],
                                                       op0=OP.add, in1=g_b[:], op1=OP.mult)
                        t2 = sml.tile([P, F], f32, tag=outname + "_t2")
                        nc.vector.scalar_tensor_tensor(out=t2[:], in0=t1[:],
                                                       scalar=rstd[:], op0=OP.mult,
                                                       in1=be_b[:], op1=OP.add)
                        hn = sml.tile([P, F], f32, tag=outname + "_hn")
                        nc.vector.tensor_scalar_max(hn[:], t2[:], 0.0)
                        return hn

                    for tt in range(NTT if lvl >= 4 else 0):
                        ph1_t = ps2.tile([P, 512], mybir.dt.float32, tag="mm512",
                                         name="ph1")
                        ph1 = ph1_t[:, :MD]
                        for c16 in range(2 * NC8):
                            if c16 < NC8:
                                lhsT = x_own[:, c16, tt * P:(tt + 1) * P]
                            else:
                                lhsT = attT[:, c16 - NC8, tt * P:(tt + 1) * P]
                            nc.tensor.matmul(ph1, lhsT=lhsT, rhs=w1_sb[:, c16, :],
                                             start=(c16 == 0), stop=(c16 == 2 * NC8 - 1))
                        h1n = layer_norm_relu(ph1, b1_sb, g1_sb, be1_sb, MD, "l1")
                        for ft in range(MD // P):
                            ptp_t = ps2.tile([P, 512], mybir.dt.float32, tag="mm512",
                                             name="ptp")
                            ptp = ptp_t[:, :P]
                            nc.tensor.transpose(ptp, h1n[:, ft * P:(ft + 1) * P], ident[:])
                            nc.vector.tensor_copy(h1T[:, ft, tt * P:(tt + 1) * P], ptp)

                    # ---------- h2 + batched LN2/final across all tiles ----------
                    hb2_all = mw.tile([P, NTT, MD2], f32, tag="hb2_all")
                    for tt in range(NTT if lvl >= 5 else 0):
                        ph2_t = ps2.tile([P, 512], mybir.dt.float32, tag="mm512",
                                         name="ph2")
                        ph2 = ph2_t[:, :MD2]
                        for ft in range(MD // P):
                            nc.tensor.matmul(ph2, lhsT=h1T[:, ft, tt * P:(tt + 1) * P],
                                             rhs=w2_sb[:, ft, :],
                                             start=(ft == 0), stop=(ft == MD // P - 1))
                        nc.vector.scalar_tensor_tensor(out=hb2_all[:, tt, :], in0=ph2,
                                                       scalar=1.0, in1=b2_sb[:],
                                                       op0=OP.mult, op1=OP.add)
                    if lvl >= 5:
                        F2 = float(MD2)
                        sums2 = sml.tile([P, NTT], f32, tag="sums2")
                        nc.vector.reduce_sum(sums2[:], hb2_all[:],
                                             axis=mybir.AxisListType.X)
                        msq = sml.tile([P, NTT, MD2], f32, tag="msq")
                        ssq2 = sml.tile([P, NTT], f32, tag="ssq2")
                        nc.vector.tensor_tensor(out=msq[:], in0=hb2_all[:],
                                                in1=hb2_all[:], op=OP.mult)
                        nc.vector.reduce_sum(ssq2[:], msq[:], axis=mybir.AxisListType.X)
                        nm2 = sml.tile([P, NTT], f32, tag="nm2")
                        nc.vector.tensor_scalar_mul(nm2[:], sums2[:], -1.0 / F2)
                        ex22 = sml.tile([P, NTT], f32, tag="ex22")
                        nc.vector.tensor_scalar_mul(ex22[:], ssq2[:], 1.0 / F2)
                        mm2 = sml.tile([P, NTT], f32, tag="mm2")
                        nc.vector.tensor_tensor(out=mm2[:], in0=nm2[:], in1=nm2[:],
                                                op=OP.mult)
                        var2 = sml.tile([P, NTT], f32, tag="var2")
                        nc.vector.tensor_tensor(out=var2[:], in0=ex22[:], in1=mm2[:],
                                                op=OP.subtract)
                        std2 = sml.tile([P, NTT], f32, tag="std2")
                        nc.scalar.activation(std2[:], var2[:], ACT.Sqrt,
                                             bias=eps_sb[:, 0:1], scale=1.0)
                        rstd2 = sml.tile([P, NTT], f32, tag="rstd2")
                        nc.vector.reciprocal(rstd2[:], std2[:])
                        t1a = sml.tile([P, NTT, MD2], f32, tag="t1a")
                        nc.vector.tensor_tensor(
                            out=t1a[:], in0=hb2_all[:],
                            in1=nm2[:, :, None].to_broadcast([P, NTT, MD2]),
                            op=OP.add)
                        nc.vector.tensor_tensor(
                            out=t1a[:], in0=t1a[:],
                            in1=rstd2[:, :, None].to_broadcast([P, NTT, MD2]),
                            op=OP.mult)
                        nc.vector.tensor_tensor(
                            out=t1a[:], in0=t1a[:],
                            in1=g2_sb[:, None, :].to_broadcast([P, NTT, MD2]),
                            op=OP.mult)
                        nc.vector.tensor_tensor(
                            out=t1a[:], in0=t1a[:],
                            in1=be2_sb[:, None, :].to_broadcast([P, NTT, MD2]),
                            op=OP.add)
                        nc.vector.tensor_scalar_max(t1a[:], t1a[:], 0.0)
                        nc.vector.tensor_tensor(
                            out=t1a[:], in0=t1a[:],
                            in1=w3_sb[:, None, :].to_broadcast([P, NTT, MD2]),
                            op=OP.mult)
                        base8 = sml.tile([P, NTT], f32, tag="base8")
                        nc.vector.reduce_sum(base8[:], t1a[:], axis=mybir.AxisListType.X)
                        nc.vector.tensor_tensor(
                            out=base8[:], in0=base8[:],
                            in1=b3_sb[:, 0:1].to_broadcast([P, NTT]), op=OP.add)
                        imp1a = sml.tile([P, NTT], f32, tag="imp1a")
                        nc.vector.tensor_scalar_add(imp1a[:], imp_all[:], 1.0)
                        nc.vector.tensor_tensor(out=base8[:], in0=base8[:],
                                                in1=imp1a[:], op=OP.mult)
                        nc.vector.tensor_scalar(base8[:], base8[:], MAX_W, MIN_W,
                                                op0=OP.min, op1=OP.max)
                        nc.vector.tensor_tensor(out=res_sb[:], in0=base8[:],
                                                in1=maskf_sb[:], op=OP.mult)
                        nc.sync.dma_start(out[:].rearrange("(t p) -> p t", p=P),
                                          res_sb[:])

    nc.compile()
    return nc


def _get_program():
    import os
    stop = os.environ.get("KB_STOP") or None
    key = ("nc", stop)
    if key not in _CACHE:
        _CACHE[key] = _build(stop)
    return _CACHE[key]


def _prep_in_maps(inputs):
    hidden = np.ascontiguousarray(np.asarray(inputs["hidden_states"], dtype=np.float32))
    token_ids = np.asarray(inputs["token_ids"], dtype=np.int32)
    mask = np.asarray(inputs["attention_mask"]).astype(bool)
    pos = np.asarray(inputs["pos_embed"], dtype=np.float32)
    in_proj_w = np.asarray(inputs["in_proj_w"], dtype=np.float32)
    in_proj_b = np.asarray(inputs["in_proj_b"], dtype=np.float32)
    out_w = np.asarray(inputs["out_w"], dtype=np.float32)
    out_b = np.asarray(inputs["out_b"], dtype=np.float32)
    w1 = np.asarray(inputs["w1"], dtype=np.float32)
    b1 = np.asarray(inputs["b1"], dtype=np.float32)
    g1 = np.asarray(inputs["g1"], dtype=np.float32)
    beta1 = np.asarray(inputs["beta1"], dtype=np.float32)
    w2 = np.asarray(inputs["w2"], dtype=np.float32)
    b2 = np.asarray(inputs["b2"], dtype=np.float32)
    g2 = np.asarray(inputs["g2"], dtype=np.float32)
    beta2 = np.asarray(inputs["beta2"], dtype=np.float32)
    w3 = np.asarray(inputs["w3"], dtype=np.float32)
    b3 = np.asarray(inputs["b3"], dtype=np.float32)
    table = np.asarray(inputs["importance_table"], dtype=np.float32)

    B, S_, H_ = hidden.shape
    assert (B, S_, H_) == (4, S, H), (B, S_, H_)

    posT = np.ascontiguousarray(pos[0].T)                      # [H, S]
    wqT = np.ascontiguousarray(in_proj_w[0:H].T)               # [H, H]
    wkT = np.ascontiguousarray(in_proj_w[H:2 * H].T)
    wvT = np.ascontiguousarray(in_proj_w[2 * H:3 * H].T)
    bq = in_proj_b[0:H]
    bk = in_proj_b[H:2 * H]
    bv = in_proj_b[2 * H:3 * H]
    owT = np.ascontiguousarray(out_w.T)
    w1T = np.ascontiguousarray(w1.T)                           # [2H, MD]
    w2T = np.ascontiguousarray(w2.T)                           # [MD, MD2]

    def cmaj(v):   # [H] -> [128, H/128] partition-major (column dt holds v[dt*128+p])
        return np.ascontiguousarray(v.reshape(-1, P).T)

    def bcast(v):  # [F] -> [128, F]
        return np.ascontiguousarray(np.broadcast_to(v[None, :], (P, v.shape[0])))

    shared = {
        "wqT": wqT, "wkT": wkT, "wvT": wvT,
        "bq_c": cmaj(bq), "bk_c": cmaj(bk), "bv_b": bcast(bv),
        "owT": owT, "ob_c": cmaj(out_b),
        "w1T": w1T, "b1_b": bcast(b1), "g1_b": bcast(g1), "be1_b": bcast(beta1),
        "w2T": w2T, "b2_b": bcast(b2), "g2_b": bcast(g2), "be2_b": bcast(beta2),
        "w3_b": bcast(w3[0]), "b3_c": np.full((P, 1), b3[0], dtype=np.float32),
        "table": np.ascontiguousarray(table[:, None]),
    }

    in_maps = []
    for c in range(8):
        b = c // 2
        half = c % 2
        own = slice(half * SQ, (half + 1) * SQ)
        oth = slice((1 - half) * SQ, (2 - half) * SQ)
        hT_b = hidden[b].T                                     # [H, S] view
        # arrange so own half occupies columns [0, SQ)
        hT_arr = np.ascontiguousarray(
            np.concatenate([hT_b[:, own], hT_b[:, oth]], axis=1))
        pT_arr = np.ascontiguousarray(
            np.concatenate([posT[:, own], posT[:, oth]], axis=1))
        kb = np.where(mask[b], 0.0, -1e9).astype(np.float32)
        kb_arr = np.concatenate([kb[own], kb[oth]])            # match column remap
        m = {
            "hT": hT_arr, "pT": pT_arr,
            "kbias": np.ascontiguousarray(kb_arr.reshape(-1, P).T),
            "maskf": np.ascontiguousarray(
                mask[b, own].astype(np.float32).reshape(-1, P).T),
            "tok": np.ascontiguousarray(token_ids[b, own][:, None]),
        }
        m.update(shared)
        in_maps.append(m)
    return in_maps


def _assemble(res):
    full = np.zeros((4, S), dtype=np.float32)
    for c in range(8):
        b = c // 2
        half = c % 2
        full[b, half * SQ:(half + 1) * SQ] = res.results[c]["out"]
    return full


def kernel(**inputs) -> np.ndarray:
    from concourse.bass_utils import run_bass_kernel_spmd
    in_maps = _prep_in_maps(inputs)
    nc = _get_program()
    res = run_bass_kernel_spmd(nc, in_maps, list(range(8)))
    return _assemble(res)


def run_traced(inputs, **kwargs):
    from concourse.bass_utils import run_bass_kernel_spmd
    in_maps = _prep_in_maps(inputs)
    nc = _get_program()
    return run_bass_kernel_spmd(nc, in_maps, list(range(8)), trace=True, **kwargs)
